# revision 45
# baseline (speedup 1.0000x reference)
"""AttentionNoPairBias on 8 Trainium2 NeuronCores.

Reference computation (B=1, S=N=2048, C=1024, H=16, DH=64), fp32:
    q = s @ Wq.T + bq ; k = k_in @ Wk.T ; v = k_in @ Wv.T
    g = sigmoid(s @ Wg.T)
    attn = softmax(q k^T / sqrt(DH) + (1-mask)*(-1e6))   per head
    out  = (g * (attn @ v)) @ Wo.T

Sharding: tensor-parallel over heads. Core c owns channels
[128c, 128(c+1)) = heads {2c, 2c+1}. Each core:
  - projects q/k/g for its 128 channels (contraction over full C,
    activations fed in transposed [C, S] layout so C sits on partitions),
  - projects v directly in [key, chan] orientation (lhsT = k_in.T chunk),
  - runs attention for its 2 heads on-chip (logits transposed
    [keys, queries]; the key mask is a per-partition bias folded into the
    Exp activation; softmax denominators come from ones-blocks in the V
    tile so the PV matmul produces them for free),
  - gates with g, writes y.T to DRAM (bf16), one AllToAll per rep
    reshards y over the sequence dim,
  - the output projection for rep i is DEFERRED by `defer` (=2) reps in
    the instruction stream (software pipelining across the repeat loop)
    so the in-order PE never stalls on the collective; a drain after
    the loop handles the still-pending reps. kt/v tiles are
    double-buffered so a new rep's projections never WAR-wait on the
    old rep's attention reads; the next rep's first kT/sT blocks are
    prefetched from the middle of the current rep.
Host concatenates the 8 row-slices.

Mask sparsity: masked keys receive softmax weight exp(-1e6) = 0 in the
reference, so the host drops them up front — k_in is compacted to the
kept keys, padded to a multiple of 256 (pad slots carry the -1e6 bias,
contributing exactly 0 to both numerator and denominator). The kernel is
compiled per (n2, jc_att) configuration and cached.

All matmul inputs are bf16 (full PE rate, halves HBM traffic vs f32);
PSUM accumulation and softmax/elementwise math are fp32.
Degenerate all-masked masks (sum == 0) are not handled.
"""

import numpy as np

B, S, N, C, H = 1, 2048, 2048, 1024, 16
DH = C // H  # 64
P = 128
NCORES = 8
CH = C // NCORES  # 128 channels per core (2 heads)
KC = C // P  # 8 contraction chunks
SBW = 512  # S-block width
NSB = S // SBW  # 4 S blocks
JC = N // P  # 16 key chunks
SROWS = S // NCORES  # 256 output rows per core
NEG = -1.0e6


PREALLOC_MM = True


def _build(n2=N, jc_att=None, repeat=1, single=False, nocc=False, defer=2):
    # n2: compacted key count (multiple of 256, <= N); jc_att: number of
    # 128-key chunks attention actually visits (trailing all-pad chunks
    # contribute exactly 0 and are skipped)
    jc_n = jc_att if jc_att is not None else n2 // P
    nbk = n2 // 256
    import concourse.mybir as mybir
    import concourse.tile as tile
    from concourse import bacc

    f32 = mybir.dt.float32
    bf16 = mybir.dt.bfloat16
    AF = mybir.ActivationFunctionType

    nc = bacc.Bacc("TRN2", target_bir_lowering=False, debug=False,
                   num_devices=(1 if single else NCORES))

    sT = nc.declare_dram_parameter("sT", [NSB, P, KC, SBW], bf16, isOutput=False)
    kT = nc.declare_dram_parameter("kT", [nbk, P, KC, 256], bf16, isOutput=False)
    wq = nc.declare_dram_parameter("wq", [P, KC, CH], bf16, isOutput=False)
    wk = nc.declare_dram_parameter("wk", [P, KC, CH], bf16, isOutput=False)
    wv = nc.declare_dram_parameter("wv", [P, KC, CH], bf16, isOutput=False)
    wg = nc.declare_dram_parameter("wg", [P, KC, CH], bf16, isOutput=False)
    bq = nc.declare_dram_parameter("bq", [CH, 1], f32, isOutput=False)
    mb = nc.declare_dram_parameter("mb", [P, jc_n], f32, isOutput=False)
    wo = nc.declare_dram_parameter("wo", [P, KC, C], bf16, isOutput=False)
    out_ext = nc.declare_dram_parameter("out", [SROWS, C], f32, isOutput=True)

    with tile.TileContext(nc) as tc:
        with (
            tc.tile_pool(name="dram", bufs=2, space="DRAM") as dpool,
            tc.tile_pool(name="dramo", bufs=defer + 1, space="DRAM") as dopool,
            tc.tile_pool(name="const", bufs=1) as cpool,
            tc.tile_pool(name="kv", bufs=2) as kvpool,
            tc.tile_pool(name="act", bufs=4) as apool,
            tc.tile_pool(name="kin", bufs=5) as kpool,
            tc.tile_pool(name="qg", bufs=2) as qgpool,
            tc.tile_pool(name="pp", bufs=6) as ppool,
            tc.tile_pool(name="yy", bufs=2) as ypool,
            tc.tile_pool(name="fin", bufs=1) as fpool,
            tc.tile_pool(name="ab", bufs=2) as abpool,
            tc.tile_pool(name="psA", bufs=3, space="PSUM") as psA,
            tc.tile_pool(name="psB", bufs=2, space="PSUM") as psB,
        ):
            # ---- constants / weights (issue order = fetch priority) ------
            halfk = KC // 2
            w_sb = {}
            t = cpool.tile([P, KC, CH], bf16, tag="wk")
            nc.sync.dma_start(t[:, 0:halfk, :], wk[:, 0:halfk, :])
            w_sb["wk"] = t
            # first key block: prefetch at top priority so matmuls start early
            in_t0 = kpool.tile([P, KC, 256], bf16, tag="kin")
            nc.sync.dma_start(in_t0[:, 0:halfk, :], kT[0, :, 0:halfk, :])
            nc.sync.dma_start(t[:, halfk:KC, :], wk[:, halfk:KC, :])
            nc.sync.dma_start(in_t0[:, halfk:KC, :], kT[0, :, halfk:KC, :])
            t = cpool.tile([P, KC, CH], bf16, tag="wv")
            nc.sync.dma_start(t[:], wv[:])
            w_sb["wv"] = t
            # first s block early: q/g projection can start while the k/v
            # projections are still streaming
            in_s0 = apool.tile([P, KC, SBW], bf16, tag="in")
            nc.sync.dma_start(in_s0[:, 0:halfk, :], sT[0, :, 0:halfk, :])
            nc.sync.dma_start(in_s0[:, halfk:KC, :], sT[0, :, halfk:KC, :])
            bq_sb = cpool.tile([CH, 1], f32, tag="bq")
            nc.sync.dma_start(bq_sb[:], bq[:])
            mb_sb = cpool.tile([P, jc_n], f32, tag="mb")
            nc.sync.dma_start(mb_sb[:], mb[:])
            ones_c = cpool.tile([P, 1], f32, tag="ones")
            nc.vector.memset(ones_c[:], 1.0)
            actwarm = cpool.tile([P, 1], f32, tag="actwarm")
            nc.scalar.activation(actwarm[:], ones_c[:], AF.Exp)
            for name, ext in (("wq", wq), ("wg", wg)):
                t = cpool.tile([P, KC, CH], bf16, tag=name)
                nc.sync.dma_start(t[:], ext[:])
                w_sb[name] = t
            wo_sb = fpool.tile([P, KC, C], bf16, tag="wo")
            nc.sync.dma_start(wo_sb[:, 0:halfk, :], wo[:, 0:halfk, :])
            nc.sync.dma_start(wo_sb[:, halfk:KC, :], wo[:, halfk:KC, :])

            def outproj_load(ccout):
                # issued ~1.5 S-blocks after the producing collective was
                # triggered, so the SP-queue wait on it is almost always
                # already satisfied (no head-of-line blocking of later DMAs)
                a_sb = abpool.tile([P, KC, SROWS], bf16, tag="a2a")
                src = ccout[:].rearrange("(kc p) i -> p kc i", p=P)
                for q in range(4):
                    nc.sync.dma_start(a_sb[:, 2 * q:2 * q + 2, :],
                                      src[:, 2 * q:2 * q + 2, :])
                return a_sb

            def outproj_mblock(a_sb, m):
                op = psA.tile([P, 2 * SBW], f32, tag="mm")
                for nb in range(2):
                    for kc in range(KC):
                        nc.tensor.matmul(
                            op[:, nb * SBW:(nb + 1) * SBW],
                            a_sb[:, kc, m * P:(m + 1) * P],
                            wo_sb[:, kc, nb * SBW:(nb + 1) * SBW],
                            start=(kc == 0), stop=(kc == KC - 1))
                o_sb = ypool.tile([P, 2 * SBW], f32, tag="osb")
                nc.vector.tensor_copy(o_sb[:], op[:])
                for q in range(2):
                    nc.sync.dma_start(
                        out_ext[m * P:(m + 1) * P, q * SBW:(q + 1) * SBW],
                        o_sb[:, q * SBW:(q + 1) * SBW])

            pending = []
            next_k0, next_k1, next_s0 = in_t0, None, in_s0
            next_mm = {}
            next_vsb = None
            for _rep in range(repeat):
                # deferred output projection, `defer` reps behind: its
                # collective has had full reps to land, so the SP-queue wait
                # below is long satisfied and never head-of-line blocks
                a_prev = (outproj_load(pending.pop(0))
                          if len(pending) >= defer else None)
                # ---- phase A: k proj [chan, key] + v proj [key, chan] ----
                kt_sb = kvpool.tile([CH, n2], bf16, tag="kt")
                # v_sb: per key chunk jc, head0 cols [v0 | ones],
                # head1 cols [ones | v1] (128+128) — the ones columns make
                # the PV matmul emit softmax denominators on free partitions.
                # The ones-memset for rep i is hoisted into rep i-1 (rolling,
                # like the input prefetches) so it never sits on the DVE
                # queue between the old rep's y-chain and the kt copy that
                # phase A is waiting for.
                if next_vsb is not None:
                    v_sb = next_vsb
                    next_vsb = None
                else:
                    v_sb = kvpool.tile([P, jc_n, 2 * P], bf16, tag="vn")
                    nc.vector.memset(v_sb[:, :, DH:2 * P - DH], 1.0)

                for jb in range(nbk):
                    if jb == 0:
                        in_t = next_k0
                    elif jb == 1 and next_k1 is not None:
                        in_t = next_k1
                    else:
                        in_t = kpool.tile([P, KC, 256], bf16, tag="kin")
                        for q in range(4):
                            nc.sync.dma_start(in_t[:, 2 * q:2 * q + 2, :],
                                              kT[jb, :, 2 * q:2 * q + 2, :])
                    ps = next_mm.pop(jb, None)
                    if ps is None:
                        ps = psA.tile([P, 2 * SBW], f32, tag="mm")
                    for kc in range(KC):
                        nc.tensor.matmul(ps[:, 0:256], w_sb["wk"][:, kc, :],
                                         in_t[:, kc, :],
                                         start=(kc == 0), stop=(kc == KC - 1))
                    # v chunks in [key, chan] orientation: lhsT = kT slice
                    for h in range(2):
                        jc = 2 * jb + h
                        if jc >= jc_n:
                            break
                        col = SBW + h * P
                        for kc in range(KC):
                            nc.tensor.matmul(
                                ps[:, col:col + P],
                                in_t[:, kc, h * P:(h + 1) * P],
                                w_sb["wv"][:, kc, :],
                                start=(kc == 0), stop=(kc == KC - 1))
                    nc.vector.tensor_copy(kt_sb[:, jb * 256:(jb + 1) * 256],
                                          ps[:, 0:256])
                    for h in range(2):
                        jc = 2 * jb + h
                        if jc >= jc_n:
                            break
                        col = SBW + h * P
                        # head0 v -> cols 0:64 ; head1 v -> cols 192:256
                        nc.scalar.copy(v_sb[:, jc, 0:DH],
                                       ps[:, col:col + DH])
                        nc.scalar.copy(v_sb[:, jc, 2 * P - DH:2 * P],
                                       ps[:, col + DH:col + 2 * DH])

                # ---- phase B: q/g proj + attention, per S block --------------
                def qg_proj(sb):
                    if sb == 0:
                        in_t = next_s0
                    else:
                        in_t = apool.tile([P, KC, SBW], bf16, tag="in")
                        for q in range(4):
                            nc.sync.dma_start(in_t[:, 2 * q:2 * q + 2, :],
                                              sT[sb, :, 2 * q:2 * q + 2, :])
                    qg = psA.tile([P, 2 * SBW], f32, tag="mm")
                    for kc in range(KC):
                        nc.tensor.matmul(qg[:, 0:SBW], w_sb["wq"][:, kc, :],
                                         in_t[:, kc, :],
                                         start=(kc == 0), stop=(kc == KC - 1))
                    for kc in range(KC):
                        nc.tensor.matmul(qg[:, SBW:2 * SBW], w_sb["wg"][:, kc, :],
                                         in_t[:, kc, :],
                                         start=(kc == 0), stop=(kc == KC - 1))
                    qt = qgpool.tile([CH, SBW], bf16, tag="qt")
                    nc.vector.tensor_add(qt[:], qg[:, 0:SBW],
                                         bq_sb[:].to_broadcast([CH, SBW]))
                    gt = qgpool.tile([CH, SBW], f32, tag="gt")
                    ge = qgpool.tile([CH, SBW], f32, tag="ge")
                    nc.scalar.activation(ge[:], qg[:, SBW:2 * SBW], AF.Exp,
                                         scale=-1.0)
                    nc.vector.tensor_scalar_add(ge[:], ge[:], 1.0)
                    nc.vector.reciprocal(gt[:], ge[:])
                    return qt, gt

                def qk_mm(qt_, jc):
                    qk = psA.tile([P, 2 * SBW], f32, tag="mm")
                    for h in range(2):
                        nc.tensor.matmul(
                            qk[:, h * SBW:(h + 1) * SBW],
                            kt_sb[h * DH:(h + 1) * DH, jc * P:(jc + 1) * P],
                            qt_[h * DH:(h + 1) * DH, :],
                            start=True, stop=True)
                    return qk

                nxt = qg_proj(0)
                qkq = []  # issued-ahead qk tiles (2-deep lookahead)
                for sb in range(NSB):
                    qt, gt = nxt

                    # attention for the 2 heads; PV accumulates over key
                    # chunks (h0 -> pv0, h1 -> pv1)
                    pv0 = psB.tile([P, SBW], f32, tag="pv")
                    pv1 = psB.tile([P, SBW], f32, tag="pv")
                    pvs = (pv0, pv1)
                    while len(qkq) < min(2, jc_n):
                        qkq.append(qk_mm(qt, len(qkq)))
                    for jc in range(jc_n):
                        pt = ppool.tile([P, 2 * SBW], bf16, tag="pt")
                        nc.scalar.activation(pt[:], qkq.pop(0)[:], AF.Exp,
                                             bias=mb_sb[:, jc:jc + 1],
                                             scale=1.0 / np.sqrt(DH))
                        nj = jc + 2
                        if nj < jc_n:
                            qkq.append(qk_mm(qt, nj))
                        elif (sb + 1 < NSB and jc_n >= 6
                              and nj - jc_n < 2):
                            # cross-block lookahead, 2 deep: the next
                            # block's first QKs fill while this block's
                            # exp tail drains, so its PVs never wait on
                            # the serial ACT exp chain
                            qkq.append(qk_mm(nxt[0], nj - jc_n))
                        for h in range(2):
                            nc.tensor.matmul(
                                pvs[h][:],
                                v_sb[:, jc, h * P:(h + 1) * P],
                                pt[:, h * SBW:(h + 1) * SBW],
                                start=(jc == 0), stop=(jc == jc_n - 1))

                        if jc == min(3, jc_n - 1) and sb + 1 < NSB:
                            # project the next block's q/g now so the next
                            # attention block starts without an ACT bubble
                            nxt = qg_proj(sb + 1)
                        # rolling prefetch of the NEXT rep's first input
                        # blocks so its phase A starts without DMA stalls
                        if _rep + 1 < repeat and jc == min(5, jc_n - 1):
                            if sb == 2:
                                next_k0 = kpool.tile([P, KC, 256], bf16,
                                                     tag="kin")
                                for q in range(4):
                                    nc.sync.dma_start(
                                        next_k0[:, 2 * q:2 * q + 2, :],
                                        kT[0, :, 2 * q:2 * q + 2, :])
                            elif sb == 3:
                                next_k1 = kpool.tile([P, KC, 256], bf16,
                                                     tag="kin")
                                for q in range(4):
                                    nc.sync.dma_start(
                                        next_k1[:, 2 * q:2 * q + 2, :],
                                        kT[1, :, 2 * q:2 * q + 2, :])
                        if (_rep + 1 < repeat and sb == 3
                                and jc == min(7, jc_n - 1)):
                            next_s0 = apool.tile([P, KC, SBW], bf16, tag="in")
                            for q in range(4):
                                nc.sync.dma_start(
                                    next_s0[:, 2 * q:2 * q + 2, :],
                                    sT[0, :, 2 * q:2 * q + 2, :])
                        if (_rep + 1 < repeat and sb == 2
                                and jc == min(2, jc_n - 1)):
                            next_vsb = kvpool.tile([P, jc_n, 2 * P], bf16,
                                                   tag="vn", name="next_vsb")
                            nc.vector.memset(
                                next_vsb[:, :, DH:2 * P - DH], 1.0)
                        # pre-allocate the next rep's first phase-A PSUM
                        # tiles while the qk ring is cool, so its matmuls
                        # don't wait for this block's exp drain
                        if PREALLOC_MM and _rep + 1 < repeat and sb == 3 \
                                and jc == 5:
                            next_mm[0] = psA.tile(
                                [P, 2 * SBW], f32, tag="mm", name="next_mm")

                    # copy o+den off PSUM immediately (releases the banks
                    # for the next S block's PV accumulators)
                    o01 = ypool.tile([P, 2 * SBW], f32, tag="o01")
                    nc.vector.tensor_copy(o01[:, 0:SBW], pv0[:])
                    nc.vector.tensor_copy(o01[:, SBW:2 * SBW], pv1[:])
                    # head0: o@rows0:64 den@64:128 ; head1: den@0:64 o@64:128
                    rec = ypool.tile([P, SBW], f32, tag="rec")
                    nc.vector.reciprocal(rec[0:DH, :], o01[DH:P, 0:SBW])
                    nc.vector.reciprocal(rec[DH:2 * DH, :],
                                         o01[0:DH, SBW:2 * SBW])
                    ytmp = ypool.tile([P, SBW], f32, tag="ytmp")
                    nc.vector.tensor_mul(ytmp[0:DH, :], o01[0:DH, 0:SBW],
                                         rec[0:DH, :])
                    nc.vector.tensor_mul(ytmp[DH:2 * DH, :],
                                         o01[P - DH:P, SBW:2 * SBW],
                                         rec[DH:2 * DH, :])
                    yt = ypool.tile([CH, SBW], bf16, tag="yt")
                    nc.vector.tensor_mul(yt[:], ytmp[:], gt[:])
                    if sb == 0:
                        cc_in = dpool.tile([NCORES * P, SROWS], bf16,
                                           tag="ccin")
                    dst = cc_in[2 * sb * P:(2 * sb + 2) * P, :].rearrange(
                        "(r p) i -> p r i", p=P)
                    nc.sync.dma_start(dst,
                                      yt[:].rearrange("p (r i) -> p r i", r=2))

                    # deferred output projection matmuls, spread over S blocks
                    if a_prev is not None and sb == 0:
                        outproj_mblock(a_prev, 0)
                    if a_prev is not None and sb == 1:
                        outproj_mblock(a_prev, 1)

                # ---- phase C: AllToAll reshard (seq-major ownership) ---------
                cc_out = dopool.tile([NCORES * P, SROWS], bf16, tag="ccout")
                if single or nocc:
                    for r in range(NCORES):
                        nc.sync.dma_start(cc_out[r * P:(r + 1) * P, :],
                                          cc_in[r * P:(r + 1) * P, :])
                else:
                    nc.gpsimd.collective_compute(
                        "AllToAll", mybir.AluOpType.bypass,
                        replica_groups=[list(range(NCORES))],
                        ins=[cc_in.opt()], outs=[cc_out.opt()])
                pending.append(cc_out)

            # ---- drain: output projections still pending -----------------
            for ccout in pending:
                a_last = outproj_load(ccout)
                outproj_mblock(a_last, 0)
                outproj_mblock(a_last, 1)

    nc.compile()
    return nc


_NC_CACHE = {}


def _n2_for(mask):
    k0 = int(np.asarray(mask).reshape(-1).astype(np.int64).sum())
    n2 = min(N, max(256, int(np.ceil(max(k0, 1) / 256.0)) * 256))
    jc_att = min(n2 // P, max(1, int(np.ceil(max(k0, 1) / P))))
    return n2, jc_att


def _get_nc(n2, jc_att):
    key = (n2, jc_att)
    if key not in _NC_CACHE:
        _NC_CACHE[key] = _build(n2=n2, jc_att=jc_att)
    return _NC_CACHE[key]


def _in_maps(inputs):
    import ml_dtypes
    bf16 = ml_dtypes.bfloat16
    s = np.asarray(inputs["s"], dtype=np.float32)
    mask = np.asarray(inputs["mask"])
    k_in = np.asarray(inputs["k_in"], dtype=np.float32)
    Wq = np.asarray(inputs["Wq"], dtype=np.float32)
    bqv = np.asarray(inputs["bq"], dtype=np.float32)
    Wk = np.asarray(inputs["Wk"], dtype=np.float32)
    Wv = np.asarray(inputs["Wv"], dtype=np.float32)
    Wg = np.asarray(inputs["Wg"], dtype=np.float32)
    Wo = np.asarray(inputs["Wo"], dtype=np.float32)

    def tile4(x2d):  # [S, C] -> [NSB, P, KC, SBW] with [sb,p,kc,n]=x2d[sb*SBW+n, kc*P+p]
        return np.ascontiguousarray(
            x2d.reshape(NSB, SBW, KC, P).transpose(0, 3, 2, 1).astype(bf16))

    def tilew(w2d):  # [C, CH] -> [P, KC, CH]
        return np.ascontiguousarray(
            w2d.reshape(KC, P, -1).transpose(1, 0, 2).astype(bf16))

    sT = tile4(s[0])

    # compact keys: keep unmasked rows, pad to a multiple of 256 with
    # slots whose bias is -1e6 (their softmax weight is exactly 0)
    n2, jc_att = _n2_for(mask)
    idx = np.flatnonzero(mask[0] != 0)[:n2]
    idx_pad = np.zeros(n2, dtype=np.int64)
    idx_pad[:len(idx)] = idx
    k_comp = np.ascontiguousarray(k_in[0][idx_pad])  # [n2, C]
    nbk = n2 // 256
    kT = np.ascontiguousarray(
        k_comp.reshape(nbk, 256, KC, P).transpose(0, 3, 2, 1).astype(bf16))
    mbias = np.full(jc_att * P, NEG, dtype=np.float32)
    mbias[:len(idx)] = 0.0
    mb_t = np.ascontiguousarray(mbias.reshape(jc_att, P).T)
    woT = np.ascontiguousarray(Wo.T)

    maps = []
    for c in range(NCORES):
        sl = slice(c * CH, (c + 1) * CH)
        maps.append({
            "sT": sT, "kT": kT,
            "wq": tilew(Wq[sl, :].T),
            "wk": tilew(Wk[sl, :].T),
            "wv": tilew(Wv[sl, :].T),
            "wg": tilew(Wg[sl, :].T),
            "bq": np.ascontiguousarray(bqv[sl].reshape(CH, 1)),
            "mb": mb_t, "wo": tilew(woT),
        })
    return maps


def _run(inputs, trace=False):
    from concourse.bass_utils import run_bass_kernel_spmd

    nc = _get_nc(*_n2_for(inputs["mask"]))
    res = run_bass_kernel_spmd(nc, _in_maps(inputs),
                               core_ids=list(range(NCORES)), trace=trace)
    full = np.concatenate([res.results[c]["out"] for c in range(NCORES)],
                          axis=0)
    return full.reshape(B, S, C).astype(np.float32), res


def kernel(**inputs) -> np.ndarray:
    out, _ = _run(inputs, trace=False)
    return out


# revision 47
# speedup vs baseline: 1.1335x; 1.1335x over previous
"""AttentionNoPairBias on 8 Trainium2 NeuronCores.

Reference computation (B=1, S=N=2048, C=1024, H=16, DH=64), fp32:
    q = s @ Wq.T + bq ; k = k_in @ Wk.T ; v = k_in @ Wv.T
    g = sigmoid(s @ Wg.T)
    attn = softmax(q k^T / sqrt(DH) + (1-mask)*(-1e6))   per head
    out  = (g * (attn @ v)) @ Wo.T

Sharding: tensor-parallel over heads. Core c owns channels
[128c, 128(c+1)) = heads {2c, 2c+1}. Each core:
  - projects q/k/g for its 128 channels (contraction over full C,
    activations fed in transposed [C, S] layout so C sits on partitions),
  - projects v directly in [key, chan] orientation (lhsT = k_in.T chunk),
  - runs attention for its 2 heads on-chip (logits transposed
    [keys, queries]; the key mask is a per-partition bias folded into the
    Exp activation; softmax denominators come from ones-blocks in the V
    tile so the PV matmul produces them for free),
  - gates with g, writes y.T to DRAM (bf16), one AllToAll per rep
    reshards y over the sequence dim,
  - the output projection for rep i is DEFERRED by `defer` (=2) reps in
    the instruction stream (software pipelining across the repeat loop)
    so the in-order PE never stalls on the collective; a drain after
    the loop handles the still-pending reps. kt/v tiles are
    double-buffered so a new rep's projections never WAR-wait on the
    old rep's attention reads; the next rep's first kT/sT blocks are
    prefetched from the middle of the current rep.
Host concatenates the 8 row-slices.

Mask sparsity: masked keys receive softmax weight exp(-1e6) = 0 in the
reference, so the host drops them up front — k_in is compacted to the
kept keys, padded to a multiple of 256 (pad slots carry the -1e6 bias,
contributing exactly 0 to both numerator and denominator). The kernel is
compiled per (n2, jc_att) configuration and cached.

All matmul inputs are bf16 (full PE rate, halves HBM traffic vs f32);
PSUM accumulation and softmax/elementwise math are fp32.
Degenerate all-masked masks (sum == 0) are not handled.
"""

import numpy as np

B, S, N, C, H = 1, 2048, 2048, 1024, 16
DH = C // H  # 64
P = 128
NCORES = 8
CH = C // NCORES  # 128 channels per core (2 heads)
KC = C // P  # 8 contraction chunks
SBW = 512  # S-block width
NSB = S // SBW  # 4 S blocks
JC = N // P  # 16 key chunks
SROWS = S // NCORES  # 256 output rows per core
NEG = -1.0e6


PREALLOC_MM = False  # phase A now uses psB tiles; mm prealloc is moot


def _build(n2=N, jc_att=None, repeat=1, single=False, nocc=False, defer=2):
    # n2: compacted key count (multiple of 256, <= N); jc_att: number of
    # 128-key chunks attention actually visits (trailing all-pad chunks
    # contribute exactly 0 and are skipped)
    jc_n = jc_att if jc_att is not None else n2 // P
    nbk = n2 // 256
    import concourse.mybir as mybir
    import concourse.tile as tile
    from concourse import bacc

    f32 = mybir.dt.float32
    bf16 = mybir.dt.bfloat16
    AF = mybir.ActivationFunctionType

    nc = bacc.Bacc("TRN2", target_bir_lowering=False, debug=False,
                   num_devices=(1 if single else NCORES))

    sT = nc.declare_dram_parameter("sT", [NSB, P, KC, SBW], bf16, isOutput=False)
    kT = nc.declare_dram_parameter("kT", [nbk, P, KC, 256], bf16, isOutput=False)
    wq = nc.declare_dram_parameter("wq", [P, KC, CH], bf16, isOutput=False)
    wk = nc.declare_dram_parameter("wk", [P, KC, CH], bf16, isOutput=False)
    wv = nc.declare_dram_parameter("wv", [P, KC, CH], bf16, isOutput=False)
    wg = nc.declare_dram_parameter("wg", [P, KC, CH], bf16, isOutput=False)
    bq = nc.declare_dram_parameter("bq", [CH, 1], f32, isOutput=False)
    mb = nc.declare_dram_parameter("mb", [P, jc_n], f32, isOutput=False)
    wo = nc.declare_dram_parameter("wo", [P, KC, C], bf16, isOutput=False)
    out_ext = nc.declare_dram_parameter("out", [SROWS, C], f32, isOutput=True)

    with tile.TileContext(nc) as tc:
        with (
            tc.tile_pool(name="dram", bufs=2, space="DRAM") as dpool,
            tc.tile_pool(name="dramo", bufs=defer + 1, space="DRAM") as dopool,
            tc.tile_pool(name="const", bufs=1) as cpool,
            tc.tile_pool(name="kv", bufs=2) as kvpool,
            tc.tile_pool(name="act", bufs=4) as apool,
            tc.tile_pool(name="kin", bufs=5) as kpool,
            tc.tile_pool(name="qg", bufs=2) as qgpool,
            tc.tile_pool(name="pp", bufs=6) as ppool,
            tc.tile_pool(name="yy", bufs=2) as ypool,
            tc.tile_pool(name="fin", bufs=1) as fpool,
            tc.tile_pool(name="ab", bufs=2) as abpool,
            tc.tile_pool(name="psA", bufs=3, space="PSUM") as psA,
            tc.tile_pool(name="psB", bufs=2, space="PSUM") as psB,
        ):
            # ---- constants / weights (issue order = fetch priority) ------
            halfk = KC // 2
            w_sb = {}
            t = cpool.tile([P, KC, CH], bf16, tag="wk")
            nc.sync.dma_start(t[:, 0:halfk, :], wk[:, 0:halfk, :])
            w_sb["wk"] = t
            # first key block: prefetch at top priority so matmuls start early
            in_t0 = kpool.tile([P, KC, 256], bf16, tag="kin")
            nc.sync.dma_start(in_t0[:, 0:halfk, :], kT[0, :, 0:halfk, :])
            nc.sync.dma_start(t[:, halfk:KC, :], wk[:, halfk:KC, :])
            nc.sync.dma_start(in_t0[:, halfk:KC, :], kT[0, :, halfk:KC, :])
            t = cpool.tile([P, KC, CH], bf16, tag="wv")
            nc.sync.dma_start(t[:], wv[:])
            w_sb["wv"] = t
            # first s block early: q/g projection can start while the k/v
            # projections are still streaming
            in_s0 = apool.tile([P, KC, SBW], bf16, tag="in")
            nc.sync.dma_start(in_s0[:, 0:halfk, :], sT[0, :, 0:halfk, :])
            nc.sync.dma_start(in_s0[:, halfk:KC, :], sT[0, :, halfk:KC, :])
            bq_sb = cpool.tile([CH, 1], f32, tag="bq")
            nc.sync.dma_start(bq_sb[:], bq[:])
            mb_sb = cpool.tile([P, jc_n], f32, tag="mb")
            nc.sync.dma_start(mb_sb[:], mb[:])
            ones_c = cpool.tile([P, 1], f32, tag="ones")
            nc.vector.memset(ones_c[:], 1.0)
            actwarm = cpool.tile([P, 1], f32, tag="actwarm")
            nc.scalar.activation(actwarm[:], ones_c[:], AF.Exp)
            for name, ext in (("wq", wq), ("wg", wg)):
                t = cpool.tile([P, KC, CH], bf16, tag=name)
                nc.sync.dma_start(t[:], ext[:])
                w_sb[name] = t
            wo_sb = fpool.tile([P, KC, C], bf16, tag="wo")
            nc.sync.dma_start(wo_sb[:, 0:halfk, :], wo[:, 0:halfk, :])
            nc.sync.dma_start(wo_sb[:, halfk:KC, :], wo[:, halfk:KC, :])

            def outproj_load(ccout):
                # issued ~1.5 S-blocks after the producing collective was
                # triggered, so the SP-queue wait on it is almost always
                # already satisfied (no head-of-line blocking of later DMAs)
                a_sb = abpool.tile([P, KC, SROWS], bf16, tag="a2a")
                src = ccout[:].rearrange("(kc p) i -> p kc i", p=P)
                for q in range(4):
                    nc.sync.dma_start(a_sb[:, 2 * q:2 * q + 2, :],
                                      src[:, 2 * q:2 * q + 2, :])
                return a_sb

            def outproj_mblock(a_sb, m):
                op = psA.tile([P, 2 * SBW], f32, tag="mm")
                for nb in range(2):
                    for kc in range(KC):
                        nc.tensor.matmul(
                            op[:, nb * SBW:(nb + 1) * SBW],
                            a_sb[:, kc, m * P:(m + 1) * P],
                            wo_sb[:, kc, nb * SBW:(nb + 1) * SBW],
                            start=(kc == 0), stop=(kc == KC - 1))
                o_sb = ypool.tile([P, 2 * SBW], f32, tag="osb")
                nc.vector.tensor_copy(o_sb[:], op[:])
                for q in range(2):
                    nc.sync.dma_start(
                        out_ext[m * P:(m + 1) * P, q * SBW:(q + 1) * SBW],
                        o_sb[:, q * SBW:(q + 1) * SBW])

            pending = []
            next_k0, next_k1, next_s0 = in_t0, None, in_s0
            next_mm = {}
            next_vsb = None
            for _rep in range(repeat):
                # deferred output projection, `defer` reps behind: its
                # collective has had full reps to land, so the SP-queue wait
                # below is long satisfied and never head-of-line blocks
                a_prev = (outproj_load(pending.pop(0))
                          if len(pending) >= defer else None)
                # ---- phase A: k proj [chan, key] + v proj [key, chan] ----
                kt_sb = kvpool.tile([CH, n2], bf16, tag="kt")
                # v_sb: per key chunk jc, head0 cols [v0 | ones],
                # head1 cols [ones | v1] (128+128) — the ones columns make
                # the PV matmul emit softmax denominators on free partitions.
                # The ones-memset for rep i is hoisted into rep i-1 (rolling,
                # like the input prefetches) so it never sits on the DVE
                # queue between the old rep's y-chain and the kt copy that
                # phase A is waiting for.
                if next_vsb is not None:
                    v_sb = next_vsb
                    next_vsb = None
                else:
                    v_sb = kvpool.tile([P, jc_n, 2 * P], bf16, tag="vn")
                    nc.vector.memset(v_sb[:, :, DH:2 * P - DH], 1.0)

                for jb in range(nbk):
                    if jb == 0:
                        in_t = next_k0
                    elif jb == 1 and next_k1 is not None:
                        in_t = next_k1
                    else:
                        in_t = kpool.tile([P, KC, 256], bf16, tag="kin")
                        for q in range(4):
                            nc.sync.dma_start(in_t[:, 2 * q:2 * q + 2, :],
                                              kT[jb, :, 2 * q:2 * q + 2, :])
                    # one [128,512] psB bank: k cols 0:256, v chunks 256:512.
                    # Using psB (freed by the o-copies right after the last
                    # PV) keeps the new rep's phase A off the psA qk ring,
                    # which is still draining through the old rep's exp tail
                    ps = psB.tile([P, SBW], f32, tag="pv")
                    for kc in range(KC):
                        nc.tensor.matmul(ps[:, 0:256], w_sb["wk"][:, kc, :],
                                         in_t[:, kc, :],
                                         start=(kc == 0), stop=(kc == KC - 1))
                    # v chunks in [key, chan] orientation: lhsT = kT slice
                    for h in range(2):
                        jc = 2 * jb + h
                        if jc >= jc_n:
                            break
                        col = 256 + h * P
                        for kc in range(KC):
                            nc.tensor.matmul(
                                ps[:, col:col + P],
                                in_t[:, kc, h * P:(h + 1) * P],
                                w_sb["wv"][:, kc, :],
                                start=(kc == 0), stop=(kc == KC - 1))
                    nc.vector.tensor_copy(kt_sb[:, jb * 256:(jb + 1) * 256],
                                          ps[:, 0:256])
                    for h in range(2):
                        jc = 2 * jb + h
                        if jc >= jc_n:
                            break
                        col = 256 + h * P
                        # head0 v -> cols 0:64 ; head1 v -> cols 192:256
                        nc.scalar.copy(v_sb[:, jc, 0:DH],
                                       ps[:, col:col + DH])
                        nc.scalar.copy(v_sb[:, jc, 2 * P - DH:2 * P],
                                       ps[:, col + DH:col + 2 * DH])

                # ---- phase B: q/g proj + attention, per S block --------------
                def qg_proj(sb):
                    if sb == 0:
                        in_t = next_s0
                    else:
                        in_t = apool.tile([P, KC, SBW], bf16, tag="in")
                        for q in range(4):
                            nc.sync.dma_start(in_t[:, 2 * q:2 * q + 2, :],
                                              sT[sb, :, 2 * q:2 * q + 2, :])
                    qg = psA.tile([P, 2 * SBW], f32, tag="mm")
                    for kc in range(KC):
                        nc.tensor.matmul(qg[:, 0:SBW], w_sb["wq"][:, kc, :],
                                         in_t[:, kc, :],
                                         start=(kc == 0), stop=(kc == KC - 1))
                    for kc in range(KC):
                        nc.tensor.matmul(qg[:, SBW:2 * SBW], w_sb["wg"][:, kc, :],
                                         in_t[:, kc, :],
                                         start=(kc == 0), stop=(kc == KC - 1))
                    qt = qgpool.tile([CH, SBW], bf16, tag="qt")
                    nc.vector.tensor_add(qt[:], qg[:, 0:SBW],
                                         bq_sb[:].to_broadcast([CH, SBW]))
                    gt = qgpool.tile([CH, SBW], f32, tag="gt")
                    ge = qgpool.tile([CH, SBW], f32, tag="ge")
                    nc.scalar.activation(ge[:], qg[:, SBW:2 * SBW], AF.Exp,
                                         scale=-1.0)
                    nc.vector.tensor_scalar_add(ge[:], ge[:], 1.0)
                    nc.vector.reciprocal(gt[:], ge[:])
                    return qt, gt

                def qk_mm(qt_, jc):
                    qk = psA.tile([P, 2 * SBW], f32, tag="mm")
                    for h in range(2):
                        nc.tensor.matmul(
                            qk[:, h * SBW:(h + 1) * SBW],
                            kt_sb[h * DH:(h + 1) * DH, jc * P:(jc + 1) * P],
                            qt_[h * DH:(h + 1) * DH, :],
                            start=True, stop=True)
                    return qk

                nxt = qg_proj(0)
                qkq = []  # issued-ahead qk tiles (2-deep lookahead)
                for sb in range(NSB):
                    qt, gt = nxt

                    # attention for the 2 heads; PV accumulates over key
                    # chunks (h0 -> pv0, h1 -> pv1)
                    pv0 = psB.tile([P, SBW], f32, tag="pv")
                    pv1 = psB.tile([P, SBW], f32, tag="pv")
                    pvs = (pv0, pv1)
                    while len(qkq) < min(2, jc_n):
                        qkq.append(qk_mm(qt, len(qkq)))
                    for jc in range(jc_n):
                        pt = ppool.tile([P, 2 * SBW], bf16, tag="pt")
                        nc.scalar.activation(pt[:], qkq.pop(0)[:], AF.Exp,
                                             bias=mb_sb[:, jc:jc + 1],
                                             scale=1.0 / np.sqrt(DH))
                        nj = jc + 2
                        if nj < jc_n:
                            qkq.append(qk_mm(qt, nj))
                        elif (sb + 1 < NSB and jc_n >= 6
                              and nj - jc_n < 2):
                            # cross-block lookahead, 2 deep: the next
                            # block's first QKs fill while this block's
                            # exp tail drains, so its PVs never wait on
                            # the serial ACT exp chain
                            qkq.append(qk_mm(nxt[0], nj - jc_n))
                        for h in range(2):
                            nc.tensor.matmul(
                                pvs[h][:],
                                v_sb[:, jc, h * P:(h + 1) * P],
                                pt[:, h * SBW:(h + 1) * SBW],
                                start=(jc == 0), stop=(jc == jc_n - 1))

                        if jc == min(3, jc_n - 1) and sb + 1 < NSB:
                            # project the next block's q/g now so the next
                            # attention block starts without an ACT bubble
                            nxt = qg_proj(sb + 1)
                        # rolling prefetch of the NEXT rep's first input
                        # blocks so its phase A starts without DMA stalls
                        if _rep + 1 < repeat and jc == min(5, jc_n - 1):
                            if sb == 2:
                                next_k0 = kpool.tile([P, KC, 256], bf16,
                                                     tag="kin")
                                for q in range(4):
                                    nc.sync.dma_start(
                                        next_k0[:, 2 * q:2 * q + 2, :],
                                        kT[0, :, 2 * q:2 * q + 2, :])
                            elif sb == 3:
                                next_k1 = kpool.tile([P, KC, 256], bf16,
                                                     tag="kin")
                                for q in range(4):
                                    nc.sync.dma_start(
                                        next_k1[:, 2 * q:2 * q + 2, :],
                                        kT[1, :, 2 * q:2 * q + 2, :])
                        if (_rep + 1 < repeat and sb == 3
                                and jc == min(7, jc_n - 1)):
                            next_s0 = apool.tile([P, KC, SBW], bf16, tag="in")
                            for q in range(4):
                                nc.sync.dma_start(
                                    next_s0[:, 2 * q:2 * q + 2, :],
                                    sT[0, :, 2 * q:2 * q + 2, :])
                        if (_rep + 1 < repeat and sb == 2
                                and jc == min(2, jc_n - 1)):
                            next_vsb = kvpool.tile([P, jc_n, 2 * P], bf16,
                                                   tag="vn", name="next_vsb")
                            nc.vector.memset(
                                next_vsb[:, :, DH:2 * P - DH], 1.0)
                        # pre-allocate the next rep's first phase-A PSUM
                        # tiles while the qk ring is cool, so its matmuls
                        # don't wait for this block's exp drain
                        if PREALLOC_MM and _rep + 1 < repeat and sb == 3 \
                                and jc == 5:
                            next_mm[0] = psA.tile(
                                [P, 2 * SBW], f32, tag="mm", name="next_mm")

                    # copy o+den off PSUM immediately (releases the banks
                    # for the next S block's PV accumulators)
                    o01 = ypool.tile([P, 2 * SBW], f32, tag="o01")
                    nc.vector.tensor_copy(o01[:, 0:SBW], pv0[:])
                    nc.vector.tensor_copy(o01[:, SBW:2 * SBW], pv1[:])
                    # head0: o@rows0:64 den@64:128 ; head1: den@0:64 o@64:128
                    rec = ypool.tile([P, SBW], f32, tag="rec")
                    nc.vector.reciprocal(rec[0:DH, :], o01[DH:P, 0:SBW])
                    nc.vector.reciprocal(rec[DH:2 * DH, :],
                                         o01[0:DH, SBW:2 * SBW])
                    ytmp = ypool.tile([P, SBW], f32, tag="ytmp")
                    nc.vector.tensor_mul(ytmp[0:DH, :], o01[0:DH, 0:SBW],
                                         rec[0:DH, :])
                    nc.vector.tensor_mul(ytmp[DH:2 * DH, :],
                                         o01[P - DH:P, SBW:2 * SBW],
                                         rec[DH:2 * DH, :])
                    yt = ypool.tile([CH, SBW], bf16, tag="yt")
                    nc.vector.tensor_mul(yt[:], ytmp[:], gt[:])
                    if sb == 0:
                        cc_in = dpool.tile([NCORES * P, SROWS], bf16,
                                           tag="ccin")
                    dst = cc_in[2 * sb * P:(2 * sb + 2) * P, :].rearrange(
                        "(r p) i -> p r i", p=P)
                    nc.sync.dma_start(dst,
                                      yt[:].rearrange("p (r i) -> p r i", r=2))

                    # deferred output projection matmuls, spread over S blocks
                    if a_prev is not None and sb == 0:
                        outproj_mblock(a_prev, 0)
                    if a_prev is not None and sb == 1:
                        outproj_mblock(a_prev, 1)

                # ---- phase C: AllToAll reshard (seq-major ownership) ---------
                cc_out = dopool.tile([NCORES * P, SROWS], bf16, tag="ccout")
                if single or nocc:
                    for r in range(NCORES):
                        nc.sync.dma_start(cc_out[r * P:(r + 1) * P, :],
                                          cc_in[r * P:(r + 1) * P, :])
                else:
                    nc.gpsimd.collective_compute(
                        "AllToAll", mybir.AluOpType.bypass,
                        replica_groups=[list(range(NCORES))],
                        ins=[cc_in.opt()], outs=[cc_out.opt()])
                pending.append(cc_out)

            # ---- drain: output projections still pending -----------------
            for ccout in pending:
                a_last = outproj_load(ccout)
                outproj_mblock(a_last, 0)
                outproj_mblock(a_last, 1)

    nc.compile()
    return nc


_NC_CACHE = {}


def _n2_for(mask):
    k0 = int(np.asarray(mask).reshape(-1).astype(np.int64).sum())
    n2 = min(N, max(256, int(np.ceil(max(k0, 1) / 256.0)) * 256))
    jc_att = min(n2 // P, max(1, int(np.ceil(max(k0, 1) / P))))
    return n2, jc_att


def _get_nc(n2, jc_att):
    key = (n2, jc_att)
    if key not in _NC_CACHE:
        _NC_CACHE[key] = _build(n2=n2, jc_att=jc_att)
    return _NC_CACHE[key]


def _in_maps(inputs):
    import ml_dtypes
    bf16 = ml_dtypes.bfloat16
    s = np.asarray(inputs["s"], dtype=np.float32)
    mask = np.asarray(inputs["mask"])
    k_in = np.asarray(inputs["k_in"], dtype=np.float32)
    Wq = np.asarray(inputs["Wq"], dtype=np.float32)
    bqv = np.asarray(inputs["bq"], dtype=np.float32)
    Wk = np.asarray(inputs["Wk"], dtype=np.float32)
    Wv = np.asarray(inputs["Wv"], dtype=np.float32)
    Wg = np.asarray(inputs["Wg"], dtype=np.float32)
    Wo = np.asarray(inputs["Wo"], dtype=np.float32)

    def tile4(x2d):  # [S, C] -> [NSB, P, KC, SBW] with [sb,p,kc,n]=x2d[sb*SBW+n, kc*P+p]
        return np.ascontiguousarray(
            x2d.reshape(NSB, SBW, KC, P).transpose(0, 3, 2, 1).astype(bf16))

    def tilew(w2d):  # [C, CH] -> [P, KC, CH]
        return np.ascontiguousarray(
            w2d.reshape(KC, P, -1).transpose(1, 0, 2).astype(bf16))

    sT = tile4(s[0])

    # compact keys: keep unmasked rows, pad to a multiple of 256 with
    # slots whose bias is -1e6 (their softmax weight is exactly 0)
    n2, jc_att = _n2_for(mask)
    idx = np.flatnonzero(mask[0] != 0)[:n2]
    idx_pad = np.zeros(n2, dtype=np.int64)
    idx_pad[:len(idx)] = idx
    k_comp = np.ascontiguousarray(k_in[0][idx_pad])  # [n2, C]
    nbk = n2 // 256
    kT = np.ascontiguousarray(
        k_comp.reshape(nbk, 256, KC, P).transpose(0, 3, 2, 1).astype(bf16))
    mbias = np.full(jc_att * P, NEG, dtype=np.float32)
    mbias[:len(idx)] = 0.0
    mb_t = np.ascontiguousarray(mbias.reshape(jc_att, P).T)
    woT = np.ascontiguousarray(Wo.T)

    maps = []
    for c in range(NCORES):
        sl = slice(c * CH, (c + 1) * CH)
        maps.append({
            "sT": sT, "kT": kT,
            "wq": tilew(Wq[sl, :].T),
            "wk": tilew(Wk[sl, :].T),
            "wv": tilew(Wv[sl, :].T),
            "wg": tilew(Wg[sl, :].T),
            "bq": np.ascontiguousarray(bqv[sl].reshape(CH, 1)),
            "mb": mb_t, "wo": tilew(woT),
        })
    return maps


def _run(inputs, trace=False):
    from concourse.bass_utils import run_bass_kernel_spmd

    nc = _get_nc(*_n2_for(inputs["mask"]))
    res = run_bass_kernel_spmd(nc, _in_maps(inputs),
                               core_ids=list(range(NCORES)), trace=trace)
    full = np.concatenate([res.results[c]["out"] for c in range(NCORES)],
                          axis=0)
    return full.reshape(B, S, C).astype(np.float32), res


def kernel(**inputs) -> np.ndarray:
    out, _ = _run(inputs, trace=False)
    return out
